# revision 23
# baseline (speedup 1.0000x reference)
"""Trainium2 Bass kernel for nn_L2LossDif (pairwise L2 contrastive loss).

Math (see the algebraic reduction in the problem's reference):
    sq_m  = sum(feats_m ** 2)           (scalar, per matrix)
    mu_m  = feats_m.sum(axis=0)         ([D], per matrix)
then a handful of scalar ops combine sq_n, sq_a, mu_n, mu_a into the loss.

Strategy: data-parallel row shard across 8 cores (1024 rows of each matrix
per core). Each core streams its 16 MiB once from HBM. Every input chunk
gets its own SBUF buffer, so the input DMAs on the sync HWDGE queue have
zero dependencies and the SDMA engines never starve — the stream runs at
the per-core HBM cap (~358 GB/s), which is the roofline here.

The chips throttle under this sustained load (periodic ~2 us dispatch
freezes on the compute engines), so per-chunk work is spread across three
engines, each well under the 2.9 us chunk period:
  - ScalarE: Square activation on cols 0:1536, accum_out -> rsq column
  - VectorE: square (tensor_tensor mult) of cols 1536:2048 elementwise-
             accumulated (tensor_add) into a [128, 512] acc per matrix;
             ScalarE reduces the acc once per matrix via an Identity
             activation with accum_out
  - TensorE: ones-matmul column sums of all 2048 cols into PSUM
The last chunk of each matrix is squared entirely by ScalarE (the final
one arrives as four 512-col pieces that pipeline behind the stream), so
VectorE is free to drain PSUM to SBUF per 512-slice right behind the stop
matmuls. Outputs are tiny and HWDGE-only: rsq [128,24] on the scalar
queue, mu [1,4096] on the sync queue, so the HBM-write receipts overlap.
Core/partition reductions + the scalar combine run on the host in float64.
"""

import numpy as np

import concourse.bacc as bacc
import concourse.mybir as mybir
import concourse.tile as tile
from concourse.bass_utils import run_bass_kernel_spmd

N_CORES = 8
N_ROWS_FULL = 8192
D = 2048
P = 128
ROWS = N_ROWS_FULL // N_CORES  # rows per core per matrix
CHUNK_ROWS = P
NCHUNK = ROWS // CHUNK_ROWS  # 1 MiB chunks per matrix
MM_N = 512  # moving free dim per matmul (one PSUM bank)
SQ_SPLIT = 1536  # squares: ScalarE takes cols :SQ_SPLIT, VectorE the rest
RSQ_COLS = 24

# rsq column layout (all written by ScalarE accum_out):
#   0..15   : per-chunk Square accums, col = m*8 + c (m1 c7 -> piece 0)
#   16,17,18: m1 last-chunk pieces 1..3
#   19      : Identity accum of VectorE's sqacc for matrix 0
#   20      : same for matrix 1
#   21..23  : zero padding
SQ_COLS = [
    list(range(0, 8)) + [19],
    list(range(8, 16)) + [16, 17, 18, 20],
]

_NC_CACHE = {}


def build_module():
    nc = bacc.Bacc("TRN2", target_bir_lowering=False, debug=False)
    f32 = mybir.dt.float32
    f32r = mybir.dt.float32r
    bf16 = mybir.dt.bfloat16
    srcs = [
        nc.dram_tensor("nfeats", [ROWS, D], f32, kind="ExternalInput"),
        nc.dram_tensor("afeats", [ROWS, D], f32, kind="ExternalInput"),
    ]
    out_mu = nc.dram_tensor("mu", [1, 2 * D], f32, kind="ExternalOutput")
    out_rsq = nc.dram_tensor("rsq", [P, RSQ_COLS], f32, kind="ExternalOutput")

    with tile.TileContext(nc) as tc:
        with (
            tc.tile_pool(name="chunks", bufs=2 * NCHUNK - 1) as chunk_pool,
            tc.tile_pool(name="last", bufs=1) as last_pool,
            tc.tile_pool(name="sq", bufs=2) as sq_pool,
            tc.tile_pool(name="vq", bufs=2) as vq_pool,
            tc.tile_pool(name="psum", bufs=1, space="PSUM") as psum_pool,
            tc.tile_pool(name="small", bufs=1) as small_pool,
        ):
            rsq_all = small_pool.tile([P, RSQ_COLS], f32)
            mu_all = small_pool.tile([1, 2 * D], f32)
            ones = small_pool.tile([P, 1], f32)
            nc.gpsimd.memset(ones, 1.0)
            nc.gpsimd.memset(rsq_all[:, RSQ_COLS - 3 :], 0.0)
            ones_r = ones.bitcast(f32r)

            def act_square(piece, col, tag):
                sq = sq_pool.tile([P, piece.shape[-1]], bf16, tag=tag)
                nc.scalar.activation(
                    out=sq,
                    in_=piece.bitcast(f32),
                    func=mybir.ActivationFunctionType.Square,
                    accum_out=rsq_all[:, col : col + 1],
                )

            # Issue every input DMA up front on the sync queue: no buffer
            # reuse, no waits — the descriptor rings stay full end to end.
            # The very last chunk arrives as four 512-col pieces so its
            # consumers start before the final bytes land.
            chunks = {}
            for m, src in enumerate(srcs):
                for c in range(NCHUNK):
                    rows = src[c * CHUNK_ROWS : (c + 1) * CHUNK_ROWS, :]
                    if m == 1 and c == NCHUNK - 1:
                        pieces = []
                        for j in range(D // MM_N):
                            p = last_pool.tile([P, MM_N], f32r, tag=f"p{j}")
                            nc.sync.dma_start(
                                out=p,
                                in_=rows[:, j * MM_N : (j + 1) * MM_N].bitcast(f32r),
                            )
                            pieces.append(p)
                        chunks[(m, c)] = pieces
                    else:
                        chunk = chunk_pool.tile([P, D], f32r)
                        nc.sync.dma_start(out=chunk, in_=rows.bitcast(f32r))
                        chunks[(m, c)] = [
                            chunk[:, j * MM_N : (j + 1) * MM_N]
                            for j in range(D // MM_N)
                        ]
                        chunks[(m, c, "full")] = chunk
                        chunks[(m, c, "lo")] = chunk[:, :SQ_SPLIT]
                        chunks[(m, c, "hi")] = chunk[:, SQ_SPLIT:]

            for m in range(2):
                psum_mu = psum_pool.tile([1, D], f32, tag=f"psum{m}")
                sqacc = small_pool.tile([P, D - SQ_SPLIT], f32, tag=f"sqacc{m}")

                def mm(j, c):
                    nc.tensor.matmul(
                        psum_mu[0:1, j * MM_N : (j + 1) * MM_N],
                        lhsT=ones_r,
                        rhs=chunks[(m, c)][j],
                        start=(c == 0),
                        stop=(c == NCHUNK - 1),
                    )

                def mu_slice(j):
                    return mu_all[0:1, m * D + j * MM_N : m * D + (j + 1) * MM_N]

                for c in range(NCHUNK):
                    last = c == NCHUNK - 1
                    if m == 1 and last:
                        # Pieces land in order; ScalarE squares each as it
                        # arrives, VectorE is free for the PSUM drain.
                        js = chunks[(m, c)]
                        act_square(js[0], SQ_COLS[1][7], "sqp")
                        act_square(js[1], 16, "sqp")
                        act_square(js[2], 17, "sqp")
                        act_square(js[3], 18, "sqp")
                        for j in range(4):
                            mm(j, c)
                    elif last:
                        # ScalarE takes the whole chunk; VectorE is done
                        # with this matrix after chunk NCHUNK-2.
                        act_square(chunks[(m, c, "full")], SQ_COLS[m][c], "sqf")
                        for j in range(4):
                            mm(j, c)
                    else:
                        act_square(chunks[(m, c, "lo")], SQ_COLS[m][c], "sq")
                        hi = chunks[(m, c, "hi")].bitcast(f32)
                        vq = vq_pool.tile([P, D - SQ_SPLIT], f32)
                        nc.vector.tensor_tensor(
                            vq, hi, hi, op=mybir.AluOpType.mult
                        )
                        if c == 0:
                            nc.vector.tensor_copy(sqacc, vq)
                        else:
                            nc.vector.tensor_add(sqacc, sqacc, vq)
                        for j in range(4):
                            mm(j, c)
                        if c == NCHUNK - 2:
                            # Free-axis reduction of the VectorE square
                            # accumulator on ScalarE (Identity activation,
                            # accum_out). sqacc is final after this chunk's
                            # add, so emitting it here keeps it off the
                            # last chunk's critical path.
                            acc_dump = sq_pool.tile(
                                [P, D - SQ_SPLIT], bf16, tag="accdump"
                            )
                            nc.scalar.activation(
                                out=acc_dump,
                                in_=sqacc,
                                func=mybir.ActivationFunctionType.Identity,
                                accum_out=rsq_all[:, 19 + m : 20 + m],
                            )
                # Drain PSUM to SBUF one 512-slice at a time: slice j is
                # final as soon as its stop matmul retires, so the copies
                # pipeline behind the matmuls.
                for j in range(D // MM_N):
                    nc.vector.tensor_copy(
                        mu_slice(j), psum_mu[0:1, j * MM_N : (j + 1) * MM_N]
                    )
            # Tiny outputs, HWDGE only: rsq rides the scalar (ACT) queue so
            # its HBM-write receipt overlaps the mu DMA's on the sync queue.
            nc.scalar.dma_start(out=out_rsq[:, :], in_=rsq_all)
            nc.sync.dma_start(out=out_mu[:, :], in_=mu_all)
    nc.compile()
    return nc


def get_module():
    if "nc" not in _NC_CACHE:
        _NC_CACHE["nc"] = build_module()
    return _NC_CACHE["nc"]


def kernel(nfeats, afeats):
    nfeats = np.asarray(nfeats, dtype=np.float32)
    afeats = np.asarray(afeats, dtype=np.float32)
    assert nfeats.shape == (N_ROWS_FULL, D) and afeats.shape == (N_ROWS_FULL, D)

    nc = get_module()
    in_maps = [
        {
            "nfeats": np.ascontiguousarray(nfeats[c * ROWS : (c + 1) * ROWS]),
            "afeats": np.ascontiguousarray(afeats[c * ROWS : (c + 1) * ROWS]),
        }
        for c in range(N_CORES)
    ]
    results = run_bass_kernel_spmd(nc, in_maps, core_ids=list(range(N_CORES))).results

    mu = np.zeros((2, D), dtype=np.float64)
    sq = np.zeros(2, dtype=np.float64)
    for r in results:
        mu += np.asarray(r["mu"], dtype=np.float64).reshape(2, D)
        rsq = np.asarray(r["rsq"], dtype=np.float64)
        for m in range(2):
            for col in SQ_COLS[m]:
                sq[m] += rsq[:, col].sum()

    return combine(mu[0], mu[1], sq[0], sq[1])


def combine(mu_n, mu_a, sq_n, sq_a):
    nnum = anum = float(N_ROWS_FULL)
    nsum = nnum * sq_n - float(mu_n @ mu_n)
    asum = anum * sq_a - float(mu_a @ mu_a)
    cross_sum = anum * sq_n + nnum * sq_a - 2.0 * float(mu_n @ mu_a)

    ncount = nnum * (nnum - 1) / 2
    acount = anum * (anum - 1) / 2
    count = nnum * anum

    loss_dif = cross_sum / count
    within = (asum + nsum) / (acount + ncount)
    loss = -np.log(loss_dif / (loss_dif + within))
    return np.asarray(loss, dtype=np.float32)
